# revision 37
# baseline (speedup 1.0000x reference)
"""Sliding-window attention + residual + LayerNorm on 8 Trainium2 NeuronCores.

Problem (hardcoded): B=1, S=4096, HID=1024, NH=16, HD=64, WIN=256.
    q,k,v = X@W* + b*  (per-head HD=64)
    scores = q k^T / 8, sliding-window mask (j in [i-128, i+128)), softmax
    out = LayerNorm(X + probs@v) * gamma + beta

Sharding: sequence-parallel. Core c owns query rows [c*512, c*512+512) and
receives X rows [c*512-128, c*512+640) (zero-padded at the sequence edges) so
all K/V it attends to are computed locally (halo recompute, no collectives).

Per-core kernel (all SBUF tiles [128 partitions, ...]):
  - X is PE-transposed once into XT [h, s] (fp32r) and reused by all three
    projections (matmuls in float32r: TF32-like, ~1.5e-4 rel err, full rate).
  - QT/KT are produced transposed ([d, s], head h = d-chunk h//2, partition
    half h%2) straight from the projection matmul as fp16; V stays natural
    [s, d] fp16 with a ones-column per head (V_aug [s, 66]) so the PV matmul
    also emits the softmax denominator Z for free.
  - scores are computed TRANSPOSED (scoresT[j, i] = kT.T @ qT) so no
    probability transpose is needed; softmax skips max-subtraction (|s| <= ~8
    cannot overflow exp; out-of-band entries are multiplied by 0 after exp,
    matching the reference's exp(-10000-max) underflow to exactly 0).
  - The whole kernel is one software pipeline over d-chunks: V projection
    rides right behind the X transposes, then for each d-chunk the Q/K
    projections are immediately followed by attention for the two heads of
    that chunk, keeping TensorE dense (and HAM-warm) while ACT/DVE do
    softmax work. LayerNorms run at the end (exp/sqrt ACT tables each load
    once).
  - kernel() specializes host-side: the 1/sqrt(HD) scale is folded into Wq,
    and zero biases / unit gamma / zero beta (as produced by setup_inputs)
    skip their ops; a generic fallback handles arbitrary values.
"""

import numpy as np

import concourse.bass as bass
import concourse.tile as tile
from concourse import bacc, mybir
from concourse import bass_utils
from concourse.masks import make_identity
from concourse.tile import add_dep_helper

F32 = mybir.dt.float32
F32R = mybir.dt.float32r
F16 = mybir.dt.float16
AFT = mybir.ActivationFunctionType

S, HID, NH, HD = 4096, 1024, 16, 64
WIN = 256
EPS = 1e-12
NCORES = 8
SLOC = S // NCORES          # 512 own rows per core
HALO = WIN // 2             # 128
KLOC = SLOC + 2 * HALO      # 768 local K/V rows
NB = SLOC // 128            # 4 query blocks per core
NKC = KLOC // 128           # 6 local K chunks
HCH = HID // 128            # 8 hidden chunks
SCALE = 1.0 / np.sqrt(HD)


def _emit(nc, tc, ctx, d, triv):
    """Emit the per-core kernel. triv: dict of bools for trivial params."""
    const = ctx.enter_context(tc.tile_pool(name="const", bufs=1))
    big = ctx.enter_context(tc.tile_pool(name="big", bufs=1))
    wres = ctx.enter_context(tc.tile_pool(name="wres", bufs=3))
    expm_p = ctx.enter_context(tc.tile_pool(name="expm", bufs=12))
    temps = ctx.enter_context(tc.tile_pool(name="temps", bufs=6))
    small = ctx.enter_context(tc.tile_pool(name="small", bufs=12))
    ctx_p = ctx.enter_context(tc.tile_pool(name="ctxp", bufs=4))
    ps1 = ctx.enter_context(tc.tile_pool(name="ps1", bufs=6, space="PSUM"))
    ps2 = ctx.enter_context(tc.tile_pool(name="ps2", bufs=1, space="PSUM"))

    # ---- constants ----
    if not triv["bq"]:
        bqs_sb = const.tile([128, HCH], F32)
        nc.sync.dma_start(out=bqs_sb,
                          in_=d["bqs"].rearrange("(c p) -> p c", p=128))
    if not triv["bk"]:
        bk_sb = const.tile([128, HCH], F32)
        nc.sync.dma_start(out=bk_sb,
                          in_=d["bk"].rearrange("(c p) -> p c", p=128))

    def bcast(src_ap):
        t = const.tile([128, HID], F32, name="bcast")
        nc.sync.dma_start(
            out=t,
            in_=bass.AP(tensor=src_ap.tensor, offset=src_ap.offset,
                        ap=[[0, 128]] + src_ap.ap))
        return t

    bv_b = None if triv["bv"] else bcast(d["bv"])
    gamma_b = None if triv["gamma"] else bcast(d["gamma"])
    beta_b = None if triv["beta"] else bcast(d["beta"])
    maskt_sb = const.tile([128, NB, 2, 128], F16)
    nc.sync.dma_start(out=maskt_sb, in_=d["maskt"].rearrange("t s j i -> j t s i"))
    ones_f = const.tile([128, 2], F32)
    nc.vector.memset(ones_f[:, 0:1], 1.0)
    nc.vector.memset(ones_f[:, 1:2], 0.0)
    ones_r = const.tile([128, 2], F16)
    nc.vector.tensor_copy(out=ones_r, in_=ones_f)

    # ---- load X (fp16): XT via DMA-transpose strips, natural for residual ----
    x_all = big.tile([128, NKC, HID], F16)
    xt_all = big.tile([128, HCH, KLOC], F16)
    dma_h = {}
    with nc.named_scope("load_x"):
        for hc in range(HCH):
            dma_h[f"xt{hc}"] = nc.sync.dma_start(
                out=xt_all[:, hc, :],
                in_=d["xh"][:, hc * 128:(hc + 1) * 128],
                transpose=True)
        for sc in range(NKC):
            dma_h[f"x{sc}"] = nc.sync.dma_start(
                out=x_all[:, sc, :], in_=d["xh"][sc * 128:(sc + 1) * 128, :])

    qt_all = big.tile([128, HCH, SLOC], F16)   # [d, dc, own s]
    kt_all = big.tile([128, HCH, KLOC], F16)   # [d, dc, local s]
    v_all = big.tile([128, NKC, NH, HD + 2], F16)  # [s, sc, head, 64+ones+pad]
    xs_sb = [ctx_p.tile([128, HID], F32, tag="xs_sb", name="xs_sb")
             for _ in range(NB)]

    def w_half(which, g):
        t = wres.tile([128, HCH, 512], F16, tag="w_half", name="w_half")
        h = nc.gpsimd.dma_start(
            out=t,
            in_=d[which].rearrange("(hc p) d -> p hc d", p=128)
            [:, :, g * 512:(g + 1) * 512])
        dma_h[f"{which}{g}"] = h
        return t

    def proj_v(g, wv_t):
        with nc.named_scope("proj_v"):
            for sc in range(NKC):
                pv = ps1.tile([128, 512], F32, tag="ps1", name="pv")
                for hc in range(HCH):
                    nc.tensor.matmul(pv,
                                     lhsT=xt_all[:, hc, sc * 128:(sc + 1) * 128],
                                     rhs=wv_t[:, hc, :],
                                     start=(hc == 0), stop=(hc == HCH - 1))
                vdst = v_all[:, sc, g * 8:(g + 1) * 8, 0:HD]
                vsrc = pv[:].rearrange("p (h e) -> p h e", e=HD)
                if triv["bv"]:
                    nc.vector.tensor_copy(out=vdst, in_=vsrc)
                else:
                    nc.vector.tensor_add(
                        out=vdst, in0=vsrc,
                        in1=bv_b[:, g * 512:(g + 1) * 512]
                        .rearrange("p (h e) -> p h e", e=HD))

    def proj_q(dc, g, wq_t):
        dci = dc - 4 * g
        with nc.named_scope("proj_q"):
            pq = ps1.tile([128, SLOC], F32, tag="ps1", name="pq")
            for hc in range(HCH):
                nc.tensor.matmul(pq, lhsT=wq_t[:, hc, dci * 128:(dci + 1) * 128],
                                 rhs=xt_all[:, hc, HALO:HALO + SLOC],
                                 start=(hc == 0), stop=(hc == HCH - 1))
            if triv["bq"]:
                nc.vector.tensor_copy(out=qt_all[:, dc, :], in_=pq)
            else:
                nc.vector.tensor_scalar(out=qt_all[:, dc, :], in0=pq,
                                        scalar1=1.0, scalar2=bqs_sb[:, dc:dc + 1],
                                        op0=mybir.AluOpType.mult,
                                        op1=mybir.AluOpType.add)

    def proj_k(dc, g, wk_t):
        dci = dc - 4 * g
        with nc.named_scope("proj_k"):
            pk = ps2.tile([128, KLOC], F32, tag="ps2", name="pk")
            for hc in range(HCH):
                nc.tensor.matmul(pk[:, 0:512],
                                 lhsT=wk_t[:, hc, dci * 128:(dci + 1) * 128],
                                 rhs=xt_all[:, hc, 0:512],
                                 start=(hc == 0), stop=(hc == HCH - 1))
                nc.tensor.matmul(pk[:, 512:KLOC],
                                 lhsT=wk_t[:, hc, dci * 128:(dci + 1) * 128],
                                 rhs=xt_all[:, hc, 512:KLOC],
                                 start=(hc == 0), stop=(hc == HCH - 1))
            if triv["bk"]:
                nc.vector.tensor_copy(out=kt_all[:, dc, :], in_=pk)
            else:
                nc.vector.tensor_scalar_add(out=kt_all[:, dc, :], in0=pk,
                                            scalar1=bk_sb[:, dc:dc + 1])

    def attention(dc):
        for t in range(NB):
            psc = {}
            with nc.named_scope("scores"):
                for c in range(3):
                    for ho in range(2):     # partition halves -> PE row tiling
                        if c == 0:
                            psc[ho] = ps1.tile([128, 3, 128], F32, tag="ps1",
                                               name="pscore")
                        ph = ho * 64
                        nc.tensor.matmul(
                            psc[ho][:, c, :],
                            lhsT=kt_all[ph:ph + 64, dc, (t + c) * 128:(t + c + 1) * 128],
                            rhs=qt_all[ph:ph + 64, dc, t * 128:(t + 1) * 128],
                            start=True, stop=True)
            cps = ps1.tile([128, 2, HD + 2], F32, tag="ps1", name="cps")
            for ho in range(2):
                h = 2 * dc + ho
                em = expm_p.tile([128, 3, 128], F16, tag="expm", name="em")
                with nc.named_scope("softmax"):
                    nc.scalar.activation(out=em, in_=psc[ho], func=AFT.Exp)
                    nc.vector.tensor_mul(
                        out=em[:, 0::2, :], in0=em[:, 0::2, :],
                        in1=maskt_sb[:, t, :, :])
                with nc.named_scope("pv"):
                    for c in range(3):
                        nc.tensor.matmul(
                            cps[:, ho, :],
                            lhsT=em[:, c, :],
                            rhs=v_all[:, t + c, h, :],
                            start=(c == 0), stop=(c == 2))
            with nc.named_scope("ctx_scale"):
                zv = small.tile([128, 2], F32, tag="zv", name="zv")
                nc.vector.reciprocal(out=zv, in_=cps[:, :, HD])
                for ho in range(2):
                    h = 2 * dc + ho
                    nc.vector.scalar_tensor_tensor(
                        out=xs_sb[t][:, h * HD:(h + 1) * HD],
                        in0=cps[:, ho, 0:HD],
                        scalar=zv[:, ho:ho + 1],
                        in1=x_all[:, t + 1, h * HD:(h + 1) * HD],
                        op0=mybir.AluOpType.mult,
                        op1=mybir.AluOpType.add)

    # ---- the pipeline ----
    for g in range(2):
        wv_t = w_half("wv", g)
        proj_v(g, wv_t)
        wq_t = w_half("wq", g)
        wk_t = w_half("wk", g)
        if g == 0:
            for sc in range(NKC):
                nc.vector.tensor_copy(
                    out=v_all[:, sc, :, HD:HD + 2],
                    in_=ones_r.unsqueeze(1).to_broadcast([128, NH, 2]))
        for dc in range(4 * g, 4 * g + 4):
            proj_q(dc, g, wq_t)
            proj_k(dc, g, wk_t)
            attention(dc)

    # ---- residual + layernorm (all blocks; phase-grouped so each ACT
    # table loads once) ----
    eps_t = const.tile([128, 1], F32)
    nc.vector.memset(eps_t, EPS)
    xs_l, mv_l, rstd_l, nmr_l = [], [], [], []
    with nc.named_scope("layernorm"):
        for t in range(NB):
            xs = xs_sb[t]
            xs_l.append(xs)
            stats = small.tile([128, 2, 6], F32, tag="stats", name="stats")
            for sg in range(2):
                nc.vector.bn_stats(out=stats[:, sg, :],
                                   in_=xs[:, sg * 512:(sg + 1) * 512])
            mv = small.tile([128, 2], F32, tag="mv", name="mv")
            nc.vector.bn_aggr(out=mv, in_=stats)
            mv_l.append(mv)
        for t in range(NB):
            rstd = small.tile([128, 1], F32, tag="rstd", name="rstd")
            nc.scalar.activation(out=rstd, in_=mv_l[t][:, 1:2], func=AFT.Sqrt,
                                 bias=eps_t)
            rstd_l.append(rstd)
        for t in range(NB):
            nc.vector.reciprocal(out=rstd_l[t], in_=rstd_l[t])
            nmr = small.tile([128, 1], F32, tag="nmr", name="nmr")
            nc.vector.tensor_scalar(out=nmr, in0=mv_l[t][:, 0:1],
                                    scalar1=rstd_l[t], scalar2=-1.0,
                                    op0=mybir.AluOpType.mult,
                                    op1=mybir.AluOpType.mult)
            nmr_l.append(nmr)
        for t in range(NB):
            xn = temps.tile([128, HID], F32, tag="xn", name="xn")
            # xn = xs*rstd - mu*rstd
            nc.scalar.activation(out=xn, in_=xs_l[t], func=AFT.Identity,
                                 bias=nmr_l[t], scale=rstd_l[t][:])
            if not triv["gamma"]:
                nc.vector.tensor_mul(out=xn, in0=xn, in1=gamma_b)
            if not triv["beta"]:
                nc.vector.tensor_add(out=xn, in0=xn, in1=beta_b)
            nc.sync.dma_start(out=d["out"][t * 128:(t + 1) * 128, :], in_=xn)


def build_module(triv):
    nc = bacc.Bacc("TRN2", target_bir_lowering=False, debug=False,
                   num_devices=NCORES)
    d = {
        "xh": nc.dram_tensor("xh", [KLOC, HID], F16, kind="ExternalInput").ap(),
        "wq": nc.dram_tensor("wq", [HID, HID], F16, kind="ExternalInput").ap(),
        "wk": nc.dram_tensor("wk", [HID, HID], F16, kind="ExternalInput").ap(),
        "wv": nc.dram_tensor("wv", [HID, HID], F16, kind="ExternalInput").ap(),
        "maskt": nc.dram_tensor("maskt", [NB, 2, 128, 128], F16,
                                kind="ExternalInput").ap(),
        "out": nc.dram_tensor("out", [SLOC, HID], F32, kind="ExternalOutput").ap(),
    }
    for nm, tv in (("bqs", "bq"), ("bk", "bk"), ("bv", "bv"),
                   ("gamma", "gamma"), ("beta", "beta")):
        if not triv[tv]:
            d[nm] = nc.dram_tensor(nm, [HID], F32, kind="ExternalInput").ap()
    from contextlib import ExitStack
    with tile.TileContext(nc) as tc:
        with ExitStack() as ctx:
            _emit(nc, tc, ctx, d, triv)
    nc.compile()
    return nc


def _make_masks():
    """maskt[core][t, side, jc, i]: 1.0 keep / 0.0 drop, scoresT orientation."""
    jc = np.arange(128)[:, None]
    i = np.arange(128)[None, :]
    band = [jc >= i, jc < i]              # side 0: chunk m=0; side 1: chunk m=2
    masks = np.zeros((NCORES, NB, 2, 128, 128), np.float32)
    for c in range(NCORES):
        for t in range(NB):
            k0 = c * SLOC + t * 128 - HALO     # global j of local chunk col 0
            for side, m in ((0, 0), (1, 2)):
                jg = k0 + m * 128 + jc
                valid = (jg >= 0) & (jg < S)
                masks[c, t, side] = (band[side] & valid).astype(np.float32)
    return masks


_STATE = {}


def kernel(**inputs):
    hs = np.asarray(inputs["hidden_states"], np.float32).reshape(S, HID)
    wq = np.asarray(inputs["Wq"], np.float32)
    wk = np.ascontiguousarray(np.asarray(inputs["Wk"], np.float16))
    wv = np.ascontiguousarray(np.asarray(inputs["Wv"], np.float16))
    bq = np.asarray(inputs["bq"], np.float32)
    bk = np.asarray(inputs["bk"], np.float32)
    bv = np.asarray(inputs["bv"], np.float32)
    gamma = np.asarray(inputs["gamma"], np.float32)
    beta = np.asarray(inputs["beta"], np.float32)

    wqs = np.ascontiguousarray((wq * np.float32(SCALE)).astype(np.float16))   # fold 1/sqrt(HD) into Wq
    triv = {
        "bq": not bq.any(), "bk": not bk.any(), "bv": not bv.any(),
        "gamma": bool(np.all(gamma == 1.0)), "beta": not beta.any(),
    }
    key = tuple(sorted(triv.items()))
    if _STATE.get("key") != key:
        _STATE["nc"] = build_module(triv)
        _STATE["key"] = key
        _STATE["masks"] = _make_masks().astype(np.float16)
    nc = _STATE["nc"]
    masks = _STATE["masks"]

    xpad = np.zeros((S + 2 * HALO, HID), np.float16)
    xpad[HALO:HALO + S] = hs.astype(np.float16)
    common = {"wq": wqs, "wk": wk, "wv": wv}
    if not triv["bq"]:
        common["bqs"] = (SCALE * bq).astype(np.float32)
    if not triv["bk"]:
        common["bk"] = bk
    if not triv["bv"]:
        common["bv"] = bv
    if not triv["gamma"]:
        common["gamma"] = gamma
    if not triv["beta"]:
        common["beta"] = beta
    in_maps = [
        {**common, "xh": np.ascontiguousarray(xpad[c * SLOC:c * SLOC + KLOC]),
         "maskt": np.ascontiguousarray(masks[c])}
        for c in range(NCORES)
    ]
    res = bass_utils.run_bass_kernel_spmd(nc, in_maps,
                                          core_ids=list(range(NCORES)),
                                          **_STATE.get("run_kwargs", {}))
    _STATE["last_result"] = res
    out = np.concatenate([res.results[c]["out"] for c in range(NCORES)], axis=0)
    return out.reshape(1, S, HID)


# revision 38
# speedup vs baseline: 1.0402x; 1.0402x over previous
"""Sliding-window attention + residual + LayerNorm on 8 Trainium2 NeuronCores.

Problem (hardcoded): B=1, S=4096, HID=1024, NH=16, HD=64, WIN=256.
    q,k,v = X@W* + b*  (per-head HD=64)
    scores = q k^T / 8, sliding-window mask (j in [i-128, i+128)), softmax
    out = LayerNorm(X + probs@v) * gamma + beta

Sharding: sequence-parallel. Core c owns query rows [c*512, c*512+512) and
receives X rows [c*512-128, c*512+640) (zero-padded at the sequence edges) so
all K/V it attends to are computed locally (halo recompute, no collectives).

Per-core kernel (all SBUF tiles [128 partitions, ...]):
  - X is PE-transposed once into XT [h, s] (fp32r) and reused by all three
    projections (matmuls in float32r: TF32-like, ~1.5e-4 rel err, full rate).
  - QT/KT are produced transposed ([d, s], head h = d-chunk h//2, partition
    half h%2) straight from the projection matmul as fp16; V stays natural
    [s, d] fp16 with a ones-column per head (V_aug [s, 66]) so the PV matmul
    also emits the softmax denominator Z for free.
  - scores are computed TRANSPOSED (scoresT[j, i] = kT.T @ qT) so no
    probability transpose is needed; softmax skips max-subtraction (|s| <= ~8
    cannot overflow exp; out-of-band entries are multiplied by 0 after exp,
    matching the reference's exp(-10000-max) underflow to exactly 0).
  - The whole kernel is one software pipeline over d-chunks: V projection
    rides right behind the X transposes, then for each d-chunk the Q/K
    projections are immediately followed by attention for the two heads of
    that chunk, keeping TensorE dense (and HAM-warm) while ACT/DVE do
    softmax work. LayerNorms run at the end (exp/sqrt ACT tables each load
    once).
  - kernel() specializes host-side: the 1/sqrt(HD) scale is folded into Wq,
    and zero biases / unit gamma / zero beta (as produced by setup_inputs)
    skip their ops; a generic fallback handles arbitrary values.
"""

import numpy as np

import concourse.bass as bass
import concourse.tile as tile
from concourse import bacc, mybir
from concourse import bass_utils
from concourse.masks import make_identity
from concourse.tile import add_dep_helper

F32 = mybir.dt.float32
F32R = mybir.dt.float32r
F16 = mybir.dt.float16
AFT = mybir.ActivationFunctionType

S, HID, NH, HD = 4096, 1024, 16, 64
WIN = 256
EPS = 1e-12
NCORES = 8
SLOC = S // NCORES          # 512 own rows per core
HALO = WIN // 2             # 128
KLOC = SLOC + 2 * HALO      # 768 local K/V rows
NB = SLOC // 128            # 4 query blocks per core
NKC = KLOC // 128           # 6 local K chunks
HCH = HID // 128            # 8 hidden chunks
SCALE = 1.0 / np.sqrt(HD)


def _emit(nc, tc, ctx, d, triv):
    """Emit the per-core kernel. triv: dict of bools for trivial params."""
    const = ctx.enter_context(tc.tile_pool(name="const", bufs=1))
    big = ctx.enter_context(tc.tile_pool(name="big", bufs=1))
    wres = ctx.enter_context(tc.tile_pool(name="wres", bufs=3))
    expm_p = ctx.enter_context(tc.tile_pool(name="expm", bufs=12))
    temps = ctx.enter_context(tc.tile_pool(name="temps", bufs=6))
    small = ctx.enter_context(tc.tile_pool(name="small", bufs=12))
    ctx_p = ctx.enter_context(tc.tile_pool(name="ctxp", bufs=4))
    ps1 = ctx.enter_context(tc.tile_pool(name="ps1", bufs=6, space="PSUM"))
    ps2 = ctx.enter_context(tc.tile_pool(name="ps2", bufs=1, space="PSUM"))

    # ---- constants ----
    if not triv["bq"]:
        bqs_sb = const.tile([128, HCH], F32)
        nc.sync.dma_start(out=bqs_sb,
                          in_=d["bqs"].rearrange("(c p) -> p c", p=128))
    if not triv["bk"]:
        bk_sb = const.tile([128, HCH], F32)
        nc.sync.dma_start(out=bk_sb,
                          in_=d["bk"].rearrange("(c p) -> p c", p=128))

    def bcast(src_ap):
        t = const.tile([128, HID], F32, name="bcast")
        nc.sync.dma_start(
            out=t,
            in_=bass.AP(tensor=src_ap.tensor, offset=src_ap.offset,
                        ap=[[0, 128]] + src_ap.ap))
        return t

    bv_b = None if triv["bv"] else bcast(d["bv"])
    gamma_b = None if triv["gamma"] else bcast(d["gamma"])
    beta_b = None if triv["beta"] else bcast(d["beta"])
    maskt_sb = const.tile([128, NB, 2, 128], F16)
    nc.sync.dma_start(out=maskt_sb, in_=d["maskt"].rearrange("t s j i -> j t s i"))
    ones_f = const.tile([128, 2], F32)
    nc.vector.memset(ones_f[:, 0:1], 1.0)
    nc.vector.memset(ones_f[:, 1:2], 0.0)
    ones_r = const.tile([128, 2], F16)
    nc.vector.tensor_copy(out=ones_r, in_=ones_f)

    # ---- load X (fp16): XT via DMA-transpose strips, natural for residual ----
    x_all = big.tile([128, NKC, HID], F16)
    xt_all = big.tile([128, HCH, KLOC], F16)
    dma_h = {}
    with nc.named_scope("load_x"):
        for hc in range(HCH):
            dma_h[f"xt{hc}"] = nc.sync.dma_start(
                out=xt_all[:, hc, :],
                in_=d["xh"][:, hc * 128:(hc + 1) * 128],
                transpose=True)
        for sc in range(NKC):
            dma_h[f"x{sc}"] = nc.sync.dma_start(
                out=x_all[:, sc, :], in_=d["xh"][sc * 128:(sc + 1) * 128, :])

    qt_all = big.tile([128, HCH, SLOC], F16)   # [d, dc, own s]
    kt_all = big.tile([128, HCH, KLOC], F16)   # [d, dc, local s]
    v_all = big.tile([128, NKC, NH, HD + 2], F16)  # [s, sc, head, 64+ones+pad]
    xs_sb = [ctx_p.tile([128, HID], F32, tag="xs_sb", name="xs_sb")
             for _ in range(NB)]

    def w_half(which, g):
        t = wres.tile([128, HCH, 512], F16, tag="w_half", name="w_half")
        h = nc.gpsimd.dma_start(
            out=t,
            in_=d[which].rearrange("(hc p) d -> p hc d", p=128)
            [:, :, g * 512:(g + 1) * 512])
        dma_h[f"{which}{g}"] = h
        return t

    def proj_v(g, wv_t):
        with nc.named_scope("proj_v"):
            for sc in range(NKC):
                pv = ps1.tile([128, 512], F32, tag="ps1", name="pv")
                for hc in range(HCH):
                    nc.tensor.matmul(pv,
                                     lhsT=xt_all[:, hc, sc * 128:(sc + 1) * 128],
                                     rhs=wv_t[:, hc, :],
                                     start=(hc == 0), stop=(hc == HCH - 1))
                vdst = v_all[:, sc, g * 8:(g + 1) * 8, 0:HD]
                vsrc = pv[:].rearrange("p (h e) -> p h e", e=HD)
                if triv["bv"]:
                    nc.vector.tensor_copy(out=vdst, in_=vsrc)
                else:
                    nc.vector.tensor_add(
                        out=vdst, in0=vsrc,
                        in1=bv_b[:, g * 512:(g + 1) * 512]
                        .rearrange("p (h e) -> p h e", e=HD))

    def proj_q(dc, g, wq_t):
        dci = dc - 4 * g
        with nc.named_scope("proj_q"):
            pq = ps1.tile([128, SLOC], F32, tag="ps1", name="pq")
            for hc in range(HCH):
                nc.tensor.matmul(pq, lhsT=wq_t[:, hc, dci * 128:(dci + 1) * 128],
                                 rhs=xt_all[:, hc, HALO:HALO + SLOC],
                                 start=(hc == 0), stop=(hc == HCH - 1))
            if triv["bq"]:
                nc.vector.tensor_copy(out=qt_all[:, dc, :], in_=pq)
            else:
                nc.vector.tensor_scalar(out=qt_all[:, dc, :], in0=pq,
                                        scalar1=1.0, scalar2=bqs_sb[:, dc:dc + 1],
                                        op0=mybir.AluOpType.mult,
                                        op1=mybir.AluOpType.add)

    def proj_k(dc, g, wk_t):
        dci = dc - 4 * g
        with nc.named_scope("proj_k"):
            pk = ps2.tile([128, KLOC], F32, tag="ps2", name="pk")
            for hc in range(HCH):
                nc.tensor.matmul(pk[:, 0:512],
                                 lhsT=wk_t[:, hc, dci * 128:(dci + 1) * 128],
                                 rhs=xt_all[:, hc, 0:512],
                                 start=(hc == 0), stop=(hc == HCH - 1))
                nc.tensor.matmul(pk[:, 512:KLOC],
                                 lhsT=wk_t[:, hc, dci * 128:(dci + 1) * 128],
                                 rhs=xt_all[:, hc, 512:KLOC],
                                 start=(hc == 0), stop=(hc == HCH - 1))
            if triv["bk"]:
                nc.vector.tensor_copy(out=kt_all[:, dc, :], in_=pk)
            else:
                nc.vector.tensor_scalar_add(out=kt_all[:, dc, :], in0=pk,
                                            scalar1=bk_sb[:, dc:dc + 1])

    def attention(dc):
        for t in range(NB):
            psc = {}
            with nc.named_scope("scores"):
                for c in range(3):
                    for ho in range(2):     # partition halves -> PE row tiling
                        if c == 0:
                            psc[ho] = ps1.tile([128, 3, 128], F32, tag="ps1",
                                               name="pscore")
                        ph = ho * 64
                        nc.tensor.matmul(
                            psc[ho][:, c, :],
                            lhsT=kt_all[ph:ph + 64, dc, (t + c) * 128:(t + c + 1) * 128],
                            rhs=qt_all[ph:ph + 64, dc, t * 128:(t + 1) * 128],
                            start=True, stop=True)
            cps = ps1.tile([128, 2, HD + 2], F32, tag="ps1", name="cps")
            for ho in range(2):
                h = 2 * dc + ho
                em = expm_p.tile([128, 3, 128], F16, tag="expm", name="em")
                with nc.named_scope("softmax"):
                    nc.scalar.activation(out=em, in_=psc[ho], func=AFT.Exp)
                    nc.vector.tensor_mul(
                        out=em[:, 0::2, :], in0=em[:, 0::2, :],
                        in1=maskt_sb[:, t, :, :])
                with nc.named_scope("pv"):
                    for c in range(3):
                        nc.tensor.matmul(
                            cps[:, ho, :],
                            lhsT=em[:, c, :],
                            rhs=v_all[:, t + c, h, :],
                            start=(c == 0), stop=(c == 2))
            with nc.named_scope("ctx_scale"):
                zv = small.tile([128, 2], F32, tag="zv", name="zv")
                nc.vector.reciprocal(out=zv, in_=cps[:, :, HD])
                for ho in range(2):
                    h = 2 * dc + ho
                    nc.vector.scalar_tensor_tensor(
                        out=xs_sb[t][:, h * HD:(h + 1) * HD],
                        in0=cps[:, ho, 0:HD],
                        scalar=zv[:, ho:ho + 1],
                        in1=x_all[:, t + 1, h * HD:(h + 1) * HD],
                        op0=mybir.AluOpType.mult,
                        op1=mybir.AluOpType.add)

    # ---- the pipeline ----
    for g in range(2):
        wv_t = w_half("wv", g)
        proj_v(g, wv_t)
        wq_t = w_half("wq", g)
        wk_t = w_half("wk", g)
        if g == 0:
            for sc in range(NKC):
                nc.vector.tensor_copy(
                    out=v_all[:, sc, :, HD:HD + 2],
                    in_=ones_r.unsqueeze(1).to_broadcast([128, NH, 2]))
        pend = None
        for dc in range(4 * g, 4 * g + 4):
            proj_q(dc, g, wq_t)
            proj_k(dc, g, wk_t)
            if pend is not None:
                attention(pend)
            pend = dc
        attention(pend)

    # ---- residual + layernorm (all blocks; phase-grouped so each ACT
    # table loads once) ----
    eps_t = const.tile([128, 1], F32)
    nc.vector.memset(eps_t, EPS)
    xs_l, mv_l, rstd_l, nmr_l = [], [], [], []
    with nc.named_scope("layernorm"):
        for t in range(NB):
            xs = xs_sb[t]
            xs_l.append(xs)
            stats = small.tile([128, 2, 6], F32, tag="stats", name="stats")
            for sg in range(2):
                nc.vector.bn_stats(out=stats[:, sg, :],
                                   in_=xs[:, sg * 512:(sg + 1) * 512])
            mv = small.tile([128, 2], F32, tag="mv", name="mv")
            nc.vector.bn_aggr(out=mv, in_=stats)
            mv_l.append(mv)
        for t in range(NB):
            rstd = small.tile([128, 1], F32, tag="rstd", name="rstd")
            nc.scalar.activation(out=rstd, in_=mv_l[t][:, 1:2], func=AFT.Sqrt,
                                 bias=eps_t)
            rstd_l.append(rstd)
        for t in range(NB):
            nc.vector.reciprocal(out=rstd_l[t], in_=rstd_l[t])
            nmr = small.tile([128, 1], F32, tag="nmr", name="nmr")
            nc.vector.tensor_scalar(out=nmr, in0=mv_l[t][:, 0:1],
                                    scalar1=rstd_l[t], scalar2=-1.0,
                                    op0=mybir.AluOpType.mult,
                                    op1=mybir.AluOpType.mult)
            nmr_l.append(nmr)
        for t in range(NB):
            xn = temps.tile([128, HID], F32, tag="xn", name="xn")
            # xn = xs*rstd - mu*rstd
            nc.scalar.activation(out=xn, in_=xs_l[t], func=AFT.Identity,
                                 bias=nmr_l[t], scale=rstd_l[t][:])
            if not triv["gamma"]:
                nc.vector.tensor_mul(out=xn, in0=xn, in1=gamma_b)
            if not triv["beta"]:
                nc.vector.tensor_add(out=xn, in0=xn, in1=beta_b)
            nc.sync.dma_start(out=d["out"][t * 128:(t + 1) * 128, :], in_=xn)


def build_module(triv):
    nc = bacc.Bacc("TRN2", target_bir_lowering=False, debug=False,
                   num_devices=NCORES)
    d = {
        "xh": nc.dram_tensor("xh", [KLOC, HID], F16, kind="ExternalInput").ap(),
        "wq": nc.dram_tensor("wq", [HID, HID], F16, kind="ExternalInput").ap(),
        "wk": nc.dram_tensor("wk", [HID, HID], F16, kind="ExternalInput").ap(),
        "wv": nc.dram_tensor("wv", [HID, HID], F16, kind="ExternalInput").ap(),
        "maskt": nc.dram_tensor("maskt", [NB, 2, 128, 128], F16,
                                kind="ExternalInput").ap(),
        "out": nc.dram_tensor("out", [SLOC, HID], F32, kind="ExternalOutput").ap(),
    }
    for nm, tv in (("bqs", "bq"), ("bk", "bk"), ("bv", "bv"),
                   ("gamma", "gamma"), ("beta", "beta")):
        if not triv[tv]:
            d[nm] = nc.dram_tensor(nm, [HID], F32, kind="ExternalInput").ap()
    from contextlib import ExitStack
    with tile.TileContext(nc) as tc:
        with ExitStack() as ctx:
            _emit(nc, tc, ctx, d, triv)
    nc.compile()
    return nc


def _make_masks():
    """maskt[core][t, side, jc, i]: 1.0 keep / 0.0 drop, scoresT orientation."""
    jc = np.arange(128)[:, None]
    i = np.arange(128)[None, :]
    band = [jc >= i, jc < i]              # side 0: chunk m=0; side 1: chunk m=2
    masks = np.zeros((NCORES, NB, 2, 128, 128), np.float32)
    for c in range(NCORES):
        for t in range(NB):
            k0 = c * SLOC + t * 128 - HALO     # global j of local chunk col 0
            for side, m in ((0, 0), (1, 2)):
                jg = k0 + m * 128 + jc
                valid = (jg >= 0) & (jg < S)
                masks[c, t, side] = (band[side] & valid).astype(np.float32)
    return masks


_STATE = {}


def kernel(**inputs):
    hs = np.asarray(inputs["hidden_states"], np.float32).reshape(S, HID)
    wq = np.asarray(inputs["Wq"], np.float32)
    wk = np.ascontiguousarray(np.asarray(inputs["Wk"], np.float16))
    wv = np.ascontiguousarray(np.asarray(inputs["Wv"], np.float16))
    bq = np.asarray(inputs["bq"], np.float32)
    bk = np.asarray(inputs["bk"], np.float32)
    bv = np.asarray(inputs["bv"], np.float32)
    gamma = np.asarray(inputs["gamma"], np.float32)
    beta = np.asarray(inputs["beta"], np.float32)

    wqs = np.ascontiguousarray((wq * np.float32(SCALE)).astype(np.float16))   # fold 1/sqrt(HD) into Wq
    triv = {
        "bq": not bq.any(), "bk": not bk.any(), "bv": not bv.any(),
        "gamma": bool(np.all(gamma == 1.0)), "beta": not beta.any(),
    }
    key = tuple(sorted(triv.items()))
    if _STATE.get("key") != key:
        _STATE["nc"] = build_module(triv)
        _STATE["key"] = key
        _STATE["masks"] = _make_masks().astype(np.float16)
    nc = _STATE["nc"]
    masks = _STATE["masks"]

    xpad = np.zeros((S + 2 * HALO, HID), np.float16)
    xpad[HALO:HALO + S] = hs.astype(np.float16)
    common = {"wq": wqs, "wk": wk, "wv": wv}
    if not triv["bq"]:
        common["bqs"] = (SCALE * bq).astype(np.float32)
    if not triv["bk"]:
        common["bk"] = bk
    if not triv["bv"]:
        common["bv"] = bv
    if not triv["gamma"]:
        common["gamma"] = gamma
    if not triv["beta"]:
        common["beta"] = beta
    in_maps = [
        {**common, "xh": np.ascontiguousarray(xpad[c * SLOC:c * SLOC + KLOC]),
         "maskt": np.ascontiguousarray(masks[c])}
        for c in range(NCORES)
    ]
    res = bass_utils.run_bass_kernel_spmd(nc, in_maps,
                                          core_ids=list(range(NCORES)),
                                          **_STATE.get("run_kwargs", {}))
    _STATE["last_result"] = res
    out = np.concatenate([res.results[c]["out"] for c in range(NCORES)], axis=0)
    return out.reshape(1, S, HID)
